# revision 47
# baseline (speedup 1.0000x reference)
"""BertSelfAttention Trainium2 Bass kernel.

B=8, S=1024, D=1024, H=16 heads, head_dim=64. Data-parallel: batch element b
runs on NeuronCore b (no collectives).

Numerics: single-pass fp16 matmuls everywhere (fp32 PSUM accumulation).
Expected rel err ~1e-4 vs the fp32 reference -- well inside the 2e-2 gate.
This is 3x less PE work than the fp16x2 split-precision scheme and 4x less
than fp32 on the A*V path (fp16 streams 1 row/cycle vs fp32's 4).

Per-core schedule (software-pipelined over d-chunks c of 128 rows = 2 heads):
  phase 1: X^T via 64 PE transposes (evac to fp16, alternating ACT/DVE)
  phase 2: V = X Wv + bv   [k, d] fp16, head-padded [k, 16*(64+2)] with ones
           columns (the ones accumulate the softmax denominator during A*V);
           bias added during PSUM evac via a broadcast bv tile (DVE add)
  phase 3: for c in 0..7:
    Q^T chunk c = Wq^T X^T + bq  [d, q] fp16 (bias fused into PSUM evac)
    K^T chunk c = Wk^T X^T + bk  [d, k] fp16
    per i (k-chunk), heads h0=2c (PE rows 0-63) and h1=2c+1 (rows 64-127)
    run concurrently:  scoresT[k, q] = K^T(h)^T Q^T(h), then one ACT exp
    per (i, h): expT = exp(scoresT/8 + mask[k]) -> fp16, FD=1024
    interleaved with pair c-1's A*V + output transform so the PE never
    waits on ACT:
      ctxT[66, q] += [V_h|1][k,:]^T expT[k, q]  (fp16, fp32 PSUM)
      per 4 q-chunks: PE-transpose ctxT -> [q, 65], DVE reciprocal of the
      denominator column, per-partition multiply, DMA head's columns out.
"""

import sys

sys.path.insert(0, "/opt/trn_rl_repo")

import numpy as np

import concourse.bass as bass  # noqa: E402
import concourse.tile as tile  # noqa: E402
from concourse import bacc, mybir  # noqa: E402
from concourse.bass import ds, ts  # noqa: E402
from concourse.bass_utils import run_bass_kernel_spmd  # noqa: E402
from concourse.masks import make_identity  # noqa: E402

B, S, D, H = 8, 1024, 1024, 16
HD = D // H  # 64
P = 128
NCH = S // P  # 8
HP = HD + 2  # 66: head block incl. ones columns
FP32 = mybir.dt.float32
FP16 = mybir.dt.float16
EXP = mybir.ActivationFunctionType.Exp

_CACHED = {}


def _build_kernel(tc):
    nc = tc.nc
    x_d = nc.dram_tensor("x", [S, D], FP32, kind="ExternalInput").ap()
    mask_d = nc.dram_tensor("mask", [S], FP32, kind="ExternalInput").ap()
    wq_d = nc.dram_tensor("Wq", [D, D], FP32, kind="ExternalInput").ap()
    bq_d = nc.dram_tensor("bq", [D], FP32, kind="ExternalInput").ap()
    wk_d = nc.dram_tensor("Wk", [D, D], FP32, kind="ExternalInput").ap()
    bk_d = nc.dram_tensor("bk", [D], FP32, kind="ExternalInput").ap()
    wv_d = nc.dram_tensor("Wv", [D, D], FP32, kind="ExternalInput").ap()
    bv_d = nc.dram_tensor("bv", [D], FP32, kind="ExternalInput").ap()
    out_d = nc.dram_tensor("out", [S, D], FP32, kind="ExternalOutput").ap()

    mm = nc.tensor.matmul

    with (
        tc.tile_pool(name="const", bufs=1) as const,
        tc.tile_pool(name="persist", bufs=1) as persist,
    ):
        identity = const.tile([P, P], FP32)
        make_identity(nc, identity[:])
        # per-partition vectors: v_sb[p, c] = vec[128c + p]. Loaded as
        # [8, 128] row tiles (8 descriptors) and PE-transposed on chip --
        # the naive "(c p) -> p c" DMA is 1024 4-byte descriptors that
        # clog the DMA rings for tens of microseconds.
        mask_sb = const.tile([P, NCH], FP32)
        bq_sb = const.tile([P, NCH], FP32)
        bk_sb = const.tile([P, NCH], FP32)
        vrow = const.tile([NCH, 3, P], FP32)
        for vi, vd in enumerate((mask_d, bq_d, bk_d)):
            nc.scalar.dma_start(
                out=vrow[:, vi, :], in_=vd.rearrange("(c p) -> c p", p=P)
            )
        bv_sb = const.tile([1, D], FP32)
        nc.scalar.dma_start(out=bv_sb[:], in_=bv_d.rearrange("(a d) -> a d", a=1))
        bv_hi = const.tile([1, D], FP16)
        nc.vector.tensor_copy(out=bv_hi[:], in_=bv_sb[:])
        ones_row = const.tile([1, P], FP16)
        nc.gpsimd.memset(ones_row[:], 1.0)
        # bv broadcast to all 128 partitions (for the V-bias add during evac)
        bv_bc = const.tile([P, D], FP32)

        xt = persist.tile([P, NCH, S], FP16, tag="xt")  # X^T: [d, s]
        v_sb = persist.tile([P, NCH, H, HP], FP16, tag="v")  # V: [k, padded d]
        nc.gpsimd.memset(v_sb[:, :, :, HD : HD + 2], 1.0)
        wqh = persist.tile([P, NCH, D], FP16, tag="wq")
        wkh = persist.tile([P, NCH, D], FP16, tag="wk")

        # front engine dedication: X^T PSUM evacs alternate ACT/DVE (both
        # DMA-paced), wv casts ride DVE right after, wq/wk casts ride ACT --
        # the V projection is gated only by the Wv DMA this way
        evac_ctr = [0]

        def xt_evac(out, in_):
            if evac_ctr[0] % 2 == 0:
                nc.scalar.copy(out=out, in_=in_)
            else:
                nc.vector.tensor_copy(out=out, in_=in_)
            evac_ctr[0] += 1

        # ---- phase 1: X^T via PE transposes ----
        # per-chunk X tiles so each chunk's transposes start as soon as its
        # own DMA lands (a single big tile serializes on the whole X load)
        with (
            tc.tile_pool(name="xpool", bufs=4) as xpool,
            tc.tile_pool(name="tpsum", bufs=4, space="PSUM") as tpsum,
        ):
            for vi, vt in enumerate((mask_sb, bq_sb, bk_sb)):
                pv = tpsum.tile([P, NCH], FP32, tag="tp", padded_shape=[P, P])
                nc.tensor.transpose(pv[:], vrow[:, vi, :], identity[0:NCH, 0:NCH])
                nc.vector.tensor_copy(out=vt[:], in_=pv[:])
            for j in range(NCH):
                x_j = xpool.tile([P, D], FP32, tag="x", name=f"x{j}")
                for rh in range(2):
                    nc.sync.dma_start(out=x_j[ts(rh, 64), :],
                                      in_=x_d[ds(j * P + rh * 64, 64), :])
                for i in range(NCH):
                    pt = tpsum.tile([P, P], FP32, tag="tp")
                    nc.tensor.transpose(pt[:], x_j[:, ts(i, P)], identity[:])
                    xt_evac(xt[:, i, ts(j, P)], pt[:])

        # ---- phase 2: V projection (and W loads/casts for q/k) ----
        with (
            tc.tile_pool(name="wstage", bufs=3) as wstage,
            tc.tile_pool(name="wvpool", bufs=1) as wvpool,
            tc.tile_pool(name="ppsum", bufs=2, space="PSUM") as ppsum,
            tc.tile_pool(name="spsum", bufs=2, space="PSUM") as spsum,
            tc.tile_pool(name="apsum", bufs=2, space="PSUM") as apsum,
            tc.tile_pool(name="qtpool", bufs=3) as qtpool,
            tc.tile_pool(name="ktpool", bufs=3) as ktpool,
            tc.tile_pool(name="exppool", bufs=4) as exppool,
            tc.tile_pool(name="ctpool", bufs=3) as ctpool,
            tc.tile_pool(name="obpool", bufs=3) as obpool,
            tc.tile_pool(name="rnpool", bufs=8) as rnpool,
        ):
            # weight DMAs ride the SP hardware queue right after X (the
            # gpsimd software-DGE path costs ~0.7us issue time per DMA and
            # paced the whole front); wv casts stay on DVE so V starts as
            # soon as Wv lands, wq/wk casts go to ACT in parallel
            wvh = wvpool.tile([P, NCH, D], FP16, tag="wv")
            for k in range(NCH):
                wt = wstage.tile([P, D], FP32, tag="wstage", name=f"wv{k}")
                for rh in range(2):
                    nc.sync.dma_start(out=wt[ts(rh, 64), :],
                                      in_=wv_d[ds(k * P + rh * 64, 64), :])
                nc.vector.tensor_copy(out=wvh[:, k], in_=wt[:])

            # bv_bc = ones^T @ bv (broadcast bias along partitions)
            for n in range(2):
                bp = ppsum.tile([P, 512], FP32, tag="proj", name=f"bvb{n}")
                mm(out=bp[:], lhsT=ones_row[:], rhs=bv_hi[:, ts(n, 512)])
                nc.vector.tensor_copy(out=bv_bc[:, ts(n, 512)], in_=bp[:])

            # V[s, d] = X Wv + bv chunk emitter, stored fp16 head-padded.
            # n-outer / k-inner: 8 consecutive MMs accumulate into one bank
            # (per-MM output-bank cycling triggers PE micro-idles that keep
            # the HAM clock-gate cold -- measured 2x slowdown)
            def emit_v_chunk(c, n):
                po = ppsum.tile([P, 512], FP32, tag="proj", name=f"v{c}_{n}")
                for k in range(NCH):
                    mm(out=po[:], lhsT=xt[:, k, ts(c, P)],
                       rhs=wvh[:, k, ts(n, 512)],
                       start=(k == 0), stop=(k == NCH - 1))
                nc.vector.tensor_tensor(
                    out=v_sb[:, c, ds(8 * n, 8), 0:HD],
                    in0=po[:].rearrange("p (h d) -> p h d", d=HD),
                    in1=bv_bc[:, ts(n, 512)].rearrange("p (h d) -> p h d", d=HD),
                    op=mybir.AluOpType.add,
                )

            # all of V up front: it fills the Wq/Wk DMA window, and keeping
            # the scores/exp pipeline undiluted at c=0 matters more than
            # filling c=0's PE idle (measured: V-in-c0 delays ACT ~8us)
            for c in range(NCH):
                for n in range(2):
                    emit_v_chunk(c, n)

            # Wq / Wk load + fp16 cast (casts on ACT: DVE is busy with V evacs)
            for w_d, wh in ((wq_d, wqh), (wk_d, wkh)):
                for k in range(NCH):
                    wt = wstage.tile([P, D], FP32, tag="wstage")
                    for rh in range(2):
                        nc.sync.dma_start(out=wt[ts(rh, 64), :],
                                          in_=w_d[ds(k * P + rh * 64, 64), :])
                    nc.scalar.copy(out=wh[:, k], in_=wt[:])

            # ---- phase 3: pipelined per-chunk attention ----
            def emit_proj(wh, b_sb, c, dst):
                for n in range(2):
                    po = ppsum.tile([P, 512], FP32, tag="proj")
                    for k in range(NCH):
                        mm(out=po[:], lhsT=wh[:, k, ts(c, P)],
                           rhs=xt[:, k, ts(n, 512)],
                           start=(k == 0), stop=(k == NCH - 1))
                    nc.vector.tensor_scalar_add(dst[:, ts(n, 512)], po[:],
                                                b_sb[:, c : c + 1])

            def emit_scores_i(c, i, qt_c, kt_c, exps):
                # heads h0 (rows 0-63) and h1 (rows 64-127) run concurrently
                # in disjoint PE row-groups (interleaved emission)
                sps = [
                    spsum.tile([P, S], FP32, tag="scores", name=f"sp{c}_{i}_{hh}")
                    for hh in range(2)
                ]
                for n in range(2):
                    for hh in range(2):
                        oh = HD * hh
                        mm(out=sps[hh][:, ts(n, 512)],
                           lhsT=kt_c[oh : oh + HD, ts(i, P)],
                           rhs=qt_c[oh : oh + HD, ts(n, 512)])
                for hh in range(2):
                    nc.scalar.activation(
                        out=exps[hh][:, i, :],
                        in_=sps[hh][:],
                        func=EXP,
                        bias=mask_sb[:, i : i + 1],
                        scale=1.0 / np.sqrt(HD).item(),
                    )

            def emit_av_chunk(h, n, expT, ct_sb):
                ctp = apsum.tile([HP, 512], FP32, tag="av", name=f"ctp{h}_{n}")
                for i in range(NCH):
                    mm(out=ctp[:], lhsT=v_sb[:, i, h, :],
                       rhs=expT[:, i, ts(n, 512)],
                       start=(i == 0), stop=(i == NCH - 1))
                nc.vector.tensor_copy(out=ct_sb[:, ts(n, 512)], in_=ctp[:])

            def emit_trans_half(h, jb, ct_sb, ob):
                # transpose 4 q-chunks, normalize, into ob; scratch shares
                # the 1-bank proj pool (temporally disjoint with QT/KT)
                ctt = ppsum.tile([P, 4 * (HD + 1)], FP32, tag="proj",
                                 name=f"ctt{h}_{jb}")
                ctt3 = ctt[:].rearrange("p (j e) -> p j e", e=HD + 1)
                for j4 in range(4):
                    j = 4 * jb + j4
                    nc.tensor.transpose(
                        ctt[:, ds((HD + 1) * j4, HD + 1)],
                        ct_sb[0 : HD + 1, ts(j, P)],
                        identity[0 : HD + 1, 0 : HD + 1],
                    )
                rn = rnpool.tile([P, 4], FP32, tag="rn")
                nc.vector.reciprocal(rn[:], ctt3[:, :, HD : HD + 1])
                for j4 in range(4):
                    nc.vector.tensor_scalar_mul(
                        ob[:, 4 * jb + j4, :], ctt3[:, j4, 0:HD],
                        rn[:, j4 : j4 + 1],
                    )

            # generator of deferred work items for pair (h0, h1): each item
            # is a closure emitting one slice of A*V / transform work.
            def av_work_items(pair):
                h0, e0, h1, e1 = pair
                items = []
                ct0 = ctpool.tile([HP, S], FP32, tag="ct", name=f"ct{h0}")
                ct1 = ctpool.tile([HP, S], FP32, tag="ct", name=f"ct{h1}")
                ob0 = obpool.tile([P, NCH, HD], FP32, tag="ob", name=f"ob{h0}")
                ob1 = obpool.tile([P, NCH, HD], FP32, tag="ob", name=f"ob{h1}")

                def dma_out(h, ob):
                    nc.sync.dma_start(
                        out=out_d[:, ds(HD * h, HD)].rearrange(
                            "(j p) d -> p j d", p=P
                        ),
                        in_=ob[:],
                    )

                def trans_dma(h, ct, ob):
                    emit_trans_half(h, 1, ct, ob)
                    dma_out(h, ob)

                items.append(lambda: emit_av_chunk(h0, 0, e0, ct0))
                items.append(lambda: emit_av_chunk(h0, 1, e0, ct0))
                items.append(lambda: emit_trans_half(h0, 0, ct0, ob0))
                items.append(lambda: trans_dma(h0, ct0, ob0))
                items.append(lambda: emit_av_chunk(h1, 0, e1, ct1))
                items.append(lambda: emit_av_chunk(h1, 1, e1, ct1))
                items.append(lambda: emit_trans_half(h1, 0, ct1, ob1))
                items.append(lambda: trans_dma(h1, ct1, ob1))
                return items

            # Q^T/K^T are produced two chunks ahead of their scores: chunks
            # 0-1 in the front (inside the weight-DMA window, so scores(0)
            # and the ACT exp pipeline start ~13us earlier) and chunk c+2
            # as interleave items during iteration c.
            qts, kts = {}, {}

            def emit_qtkt(c):
                qts[c] = qtpool.tile([P, S], FP16, tag="qt", name=f"qt{c}")
                kts[c] = ktpool.tile([P, S], FP16, tag="kt", name=f"kt{c}")
                emit_proj(wqh, bq_sb, c, qts[c])
                emit_proj(wkh, bk_sb, c, kts[c])

            emit_qtkt(0)
            emit_qtkt(1)

            prev_items = []
            for c in range(NCH):
                e0 = exppool.tile([P, NCH, S], FP16, tag="exp", name=f"e{2 * c}")
                e1 = exppool.tile([P, NCH, S], FP16, tag="exp",
                                  name=f"e{2 * c + 1}")
                items = list(prev_items)
                if c + 2 < NCH:
                    items.insert(min(2, len(items)),
                                 (lambda cc: (lambda: emit_qtkt(cc)))(c + 2))
                # interleave scores chunks with deferred work (prev pair's
                # AV/transform, the c+2 projections) so the PE stays busy
                # while ACT works through the exps
                for i in range(NCH):
                    emit_scores_i(c, i, qts[c], kts[c], (e0, e1))
                    if items:
                        items.pop(0)()
                    if i == NCH - 1:
                        while items:
                            items.pop(0)()
                qts.pop(c), kts.pop(c)
                prev_items = av_work_items((2 * c, e0, 2 * c + 1, e1))
            while prev_items:
                prev_items.pop(0)()


def _ensure_ntff_hook():
    """antenv.axon_hooks is absent in this image; recreate it so
    run_bass_kernel_spmd(trace=True) can capture NTFF profiles."""
    import types

    try:
        from antenv.axon_hooks import get_axon_ntff_profile_hook  # noqa: F401

        return
    except ImportError:
        pass
    from trn_agent_boot.trn_boot import _ntff_profile_via_ctypes

    hook = _ntff_profile_via_ctypes("/opt/axon/libaxon_pjrt.so")
    mod = types.ModuleType("antenv.axon_hooks")
    mod._hook = hook
    mod.get_axon_ntff_profile_hook = lambda: mod._hook
    mod.set_axon_ntff_profile_hook = lambda h: setattr(mod, "_hook", h)
    sys.modules["antenv.axon_hooks"] = mod


def _get_compiled():
    if "nc" not in _CACHED:
        nc = bacc.Bacc(
            "TRN2", target_bir_lowering=False, debug=False, num_devices=B
        )
        with tile.TileContext(nc) as tc:
            _build_kernel(tc)
        nc.compile()
        _CACHED["nc"] = nc
    return _CACHED["nc"]


def kernel(hidden_states, attention_mask, Wq, bq, Wk, bk, Wv, bv, **run_kwargs):
    hs = np.ascontiguousarray(np.asarray(hidden_states, dtype=np.float32))
    am = np.ascontiguousarray(np.asarray(attention_mask, dtype=np.float32)).reshape(B, S)
    weights = {
        "Wq": np.ascontiguousarray(np.asarray(Wq, dtype=np.float32)),
        "bq": np.ascontiguousarray(np.asarray(bq, dtype=np.float32)),
        "Wk": np.ascontiguousarray(np.asarray(Wk, dtype=np.float32)),
        "bk": np.ascontiguousarray(np.asarray(bk, dtype=np.float32)),
        "Wv": np.ascontiguousarray(np.asarray(Wv, dtype=np.float32)),
        "bv": np.ascontiguousarray(np.asarray(bv, dtype=np.float32)),
    }
    if run_kwargs.get("trace"):
        _ensure_ntff_hook()
    nc = _get_compiled()
    in_maps = [
        {"x": hs[b], "mask": am[b], **weights} for b in range(B)
    ]
    res = run_bass_kernel_spmd(nc, in_maps, core_ids=list(range(B)), **run_kwargs)
    out = np.stack([res.results[b]["out"] for b in range(B)], axis=0)
    if run_kwargs:
        kernel.last_results = res
    return out


if __name__ == "__main__":
    rng = np.random.default_rng(0)
    inputs = {
        "hidden_states": rng.standard_normal((B, S, D), dtype=np.float32),
        "attention_mask": np.zeros((B, 1, 1, S), dtype=np.float32),
        "Wq": rng.standard_normal((D, D), dtype=np.float32) / 32.0,
        "bq": rng.standard_normal(D, dtype=np.float32) * 0.02,
        "Wk": rng.standard_normal((D, D), dtype=np.float32) / 32.0,
        "bk": rng.standard_normal(D, dtype=np.float32) * 0.02,
        "Wv": rng.standard_normal((D, D), dtype=np.float32) / 32.0,
        "bv": rng.standard_normal(D, dtype=np.float32) * 0.02,
    }
    out = kernel(**inputs)
    print("out", out.shape, out.dtype, float(np.abs(out).mean()))
